# revision 20
# baseline (speedup 1.0000x reference)
"""Fused masked-attention kernel for Trainium2, data-parallel over batch on 8 cores.

Per core (one batch element): computes
  Q = query @ WQ.T ; K = key @ WK.T ; V = value @ WV.T      (H=64)
  S^T[k,q] = (K Q^T)[k,q]  (scores transposed, k on partitions)
  P^T = exp(S^T * 0.125) * notmask^T
  O_ext^T[h,q] = sum_k V_ext[k,h] P^T[k,q]   (V_ext has a ones column -> row 64 = Z)
  out[q,h] = O^T[h,q] / Z[q]   (via PE transpose + per-partition scalar mult)

Inputs are host-transposed (qT/kT/vT [E,L], inverted mask notmT [Lk,Lq]) so all
on-chip matmuls have their contraction dim on partitions with zero on-chip
transposes of large tensors. f32/u8 inputs are cast to fp16 during the (SWDGE)
DMA so every matmul runs at full PE rate; accumulation stays f32 in PSUM and
the softmax normalization stays f32. Q^T/K^T are zero-padded to K=128 so the
score matmuls drive all 128 PE rows (keeps the HAM activity monitor warm).
exp/mask-mult run 1024 wide (two 512-wide score tiles in adjacent PSUM banks)
to amortize per-instruction overhead on ACT/DVE.
"""

import numpy as np

import concourse.bass as bass
import concourse.tile as tile
from concourse import bacc, mybir
from concourse import bass_utils

B, L, E, H = 8, 4096, 1024, 64
NCORES = 8
F32 = mybir.dt.float32
F16 = mybir.dt.float16
U8 = mybir.dt.uint8

LB = 512  # l-block (free dim) for projections and q-blocks


def build_nc():
    nc = bacc.Bacc(
        "TRN2",
        target_bir_lowering=False,
        debug=False,
        enable_asserts=False,
        num_devices=NCORES,
    )
    qT = nc.dram_tensor("qT", [E, L], F32, kind="ExternalInput").ap()
    kT = nc.dram_tensor("kT", [E, L], F32, kind="ExternalInput").ap()
    vT = nc.dram_tensor("vT", [E, L], F32, kind="ExternalInput").ap()
    notmT = nc.dram_tensor("notmT", [L, L], U8, kind="ExternalInput").ap()
    wqT = nc.dram_tensor("wqT", [E, H], F32, kind="ExternalInput").ap()
    wkT = nc.dram_tensor("wkT", [E, H], F32, kind="ExternalInput").ap()
    wvT = nc.dram_tensor("wvT", [E, H], F32, kind="ExternalInput").ap()
    ident = nc.dram_tensor("ident", [128, 128], F32, kind="ExternalInput").ap()
    out = nc.dram_tensor("out", [L, H], F32, kind="ExternalOutput").ap()

    EXP = mybir.ActivationFunctionType.Exp
    NQB = L // LB

    qT_r = qT.rearrange("(c p) l -> p c l", p=128)
    kT_r = kT.rearrange("(c p) l -> p c l", p=128)
    vT_r = vT.rearrange("(c p) l -> p c l", p=128)
    notmT_r = notmT.rearrange("(c p) q -> p c q", p=128)

    with tile.TileContext(nc) as tc:
        with (
            tc.tile_pool(name="const", bufs=1) as constp,
            tc.tile_pool(name="persist", bufs=1) as persist,
            tc.tile_pool(name="kin", bufs=6) as kinp,
            tc.tile_pool(name="vin", bufs=4) as vinp,
            tc.tile_pool(name="qin", bufs=3) as qinp,
            tc.tile_pool(name="mask", bufs=2) as mpool,
            tc.tile_pool(name="pt", bufs=4) as ptpool,
            tc.tile_pool(name="osb", bufs=2) as opool,
            tc.tile_pool(name="zinv", bufs=4) as zpool,
            tc.tile_pool(name="otile", bufs=4) as otpool,
            tc.tile_pool(name="ps_st", bufs=2, space="PSUM") as ps_st,
            tc.tile_pool(name="ps_o", bufs=1, space="PSUM") as ps_o,
            tc.tile_pool(name="ps_small", bufs=3, space="PSUM") as ps_small,
        ):
            ident_sb = constp.tile([128, 128], F32)
            nc.sync.dma_start(ident_sb[:], ident)
            # weights, e-chunked: [128, 8, 64], cast to fp16 during DMA
            wq_sb = constp.tile([128, 8, H], F16)
            wk_sb = constp.tile([128, 8, H], F16)
            wv_sb = constp.tile([128, 8, H], F16)
            nc.gpsimd.dma_start(wq_sb[:], wqT.rearrange("(c p) h -> p c h", p=128))
            nc.gpsimd.dma_start(wk_sb[:], wkT.rearrange("(c p) h -> p c h", p=128))
            nc.gpsimd.dma_start(wv_sb[:], wvT.rearrange("(c p) h -> p c h", p=128))

            # Q^T/K^T [h, l] zero-padded to 128 rows (full-row score matmuls)
            QT_sb = persist.tile([128, L], F16)
            KT_sb = persist.tile([128, L], F16)
            nc.vector.memset(QT_sb[64:128, :], 0.0)
            nc.vector.memset(KT_sb[64:128, :], 0.0)
            V_sb = persist.tile([128, 32, H + 1], F16)  # V [k, h] + ones col
            nc.vector.memset(V_sb[:, :, H : H + 1], 1.0)

            # ---------------- Phase 1: K/V loads + projections ----------------
            # All of K first so the first score matmuls can start while V is
            # still loading/projecting.
            for lb in range(L // LB):
                ls = lb * LB
                k_in = kinp.tile([128, 8, LB], F16, tag="kin")
                nc.gpsimd.dma_start(k_in[:], kT_r[:, :, ls : ls + LB])
                p_kt = ps_small.tile([64, LB], F32, tag="small")
                for ec in range(8):
                    nc.tensor.matmul(
                        p_kt[:], wk_sb[:, ec, :], k_in[:, ec, :],
                        start=(ec == 0), stop=(ec == 7),
                    )
                nc.scalar.copy(KT_sb[0:64, ls : ls + LB], p_kt[:])
            for lb in range(L // LB):
                ls = lb * LB
                v_in = vinp.tile([128, 8, LB], F16, tag="vin")
                nc.gpsimd.dma_start(v_in[:], vT_r[:, :, ls : ls + LB])
                # V: [k,h] layout -> stationary = vT chunk, moving = wvT chunk
                for sub in range(LB // 128):
                    p_v = ps_small.tile([128, H], F32, tag="small")
                    for ec in range(8):
                        nc.tensor.matmul(
                            p_v[:],
                            v_in[:, ec, sub * 128 : (sub + 1) * 128],
                            wv_sb[:, ec, :],
                            start=(ec == 0),
                            stop=(ec == 7),
                        )
                    nc.scalar.copy(V_sb[:, lb * 4 + sub, 0:H], p_v[:])

            # ---------------- Phase 2: Q proj + scores/softmax/AV ----------------
            def load_q(qb):
                qs = qb * LB
                q_in = qinp.tile([128, 8, LB], F16, tag="qin")
                nc.gpsimd.dma_start(q_in[:], qT_r[:, :, qs : qs + LB])
                return q_in

            def proj_q(qb, q_in):
                qs = qb * LB
                p_qt = ps_small.tile([64, LB], F32, tag="small")
                for ec in range(8):
                    nc.tensor.matmul(
                        p_qt[:], wq_sb[:, ec, :], q_in[:, ec, :],
                        start=(ec == 0), stop=(ec == 7),
                    )
                nc.scalar.copy(QT_sb[0:64, qs : qs + LB], p_qt[:])

            def load_mask(qb):
                qs = qb * LB
                mtile = mpool.tile([128, 32 * LB], U8, tag="m")
                nc.sync.dma_start(
                    mtile[:].rearrange("p (c q) -> p c q", q=LB),
                    notmT_r[:, :, qs : qs + LB],
                )
                return mtile

            def epilogue(qb, p_o):
                qs = qb * LB
                o_sb = opool.tile([H + 1, LB], F32, tag="o_sb")
                nc.scalar.copy(o_sb[:], p_o[:])
                for sub in range(LB // 128):
                    p_t = ps_small.tile([128, H + 1], F32, tag="small")
                    nc.tensor.transpose(
                        p_t[:],
                        o_sb[:, sub * 128 : (sub + 1) * 128],
                        ident_sb[0 : H + 1, 0 : H + 1],
                    )
                    zinv = zpool.tile([128, 1], F32, tag="zinv")
                    nc.vector.reciprocal(zinv[:], p_t[:, H : H + 1])
                    ot = otpool.tile([128, H], F32, tag="ot")
                    nc.vector.tensor_scalar_mul(ot[:], p_t[:, 0:H], zinv[:])
                    r0 = qs + sub * 128
                    nc.sync.dma_start(out[r0 : r0 + 128, :], ot[:])

            # prologue: q/mask for block 0, q for block 1 (overlaps phase 1)
            q0 = load_q(0)
            m0 = load_mask(0)
            proj_q(0, q0)
            q_next = load_q(1)

            # dense PE warmup burst to trip the HAM SHORT busy window
            p_w = ps_st.tile([128, 128], F32, tag="p_st")
            for w in range(48):
                nc.tensor.matmul(
                    p_w[:], KT_sb[:, 0:128], KT_sb[:, 0:128],
                    start=True, stop=True,
                )

            pending = None  # (qb, p_o) awaiting epilogue
            mtile = m0
            for qb in range(NQB):
                qs = qb * LB
                p_o = ps_o.tile([H + 1, LB], F32, tag="p_o")
                m_next = None
                for g in range(8):  # groups of 4 k-chunks (2 wide pairs)
                    wides = []
                    for j in range(2):
                        kc = 4 * g + 2 * j
                        p_st = ps_st.tile([128, 2 * LB], F32, tag="p_st")
                        nc.tensor.matmul(
                            p_st[:, 0:LB],
                            KT_sb[:, kc * 128 : (kc + 1) * 128],
                            QT_sb[:, qs : qs + LB],
                            start=True,
                            stop=True,
                        )
                        nc.tensor.matmul(
                            p_st[:, LB : 2 * LB],
                            KT_sb[:, (kc + 1) * 128 : (kc + 2) * 128],
                            QT_sb[:, qs : qs + LB],
                            start=True,
                            stop=True,
                        )
                        wides.append((kc, p_st))
                    if g == 1 and pending is not None:
                        epilogue(*pending)
                        pending = None
                    if g == 2 and qb + 1 < NQB:
                        m_next = load_mask(qb + 1)
                    if g == 4 and qb + 1 < NQB:
                        proj_q(qb + 1, q_next)
                    if g == 6 and qb + 2 < NQB:
                        q_next = load_q(qb + 2)
                    pts = []
                    for kc, p_st in wides:
                        pt = ptpool.tile([128, 2 * LB], F16, tag="pt")
                        nc.scalar.activation(pt[:], p_st[:], EXP, scale=0.125)
                        nc.vector.tensor_mul(
                            pt[:], pt[:], mtile[:, kc * LB : (kc + 2) * LB]
                        )
                        pts.append((kc, pt))
                    for kc, pt in pts:
                        nc.tensor.matmul(
                            p_o[:], V_sb[:, kc, :], pt[:, 0:LB],
                            start=(kc == 0), stop=False,
                        )
                        nc.tensor.matmul(
                            p_o[:], V_sb[:, kc + 1, :], pt[:, LB : 2 * LB],
                            start=False, stop=(kc + 1 == 31),
                        )
                pending = (qb, p_o)
                mtile = m_next
            epilogue(*pending)
    nc.compile()
    return nc


_NC_CACHE = {}


def kernel(query, key, value, mask, WQ, WK, WV):
    if "nc" not in _NC_CACHE:
        _NC_CACHE["nc"] = build_nc()
    nc = _NC_CACHE["nc"]

    ident = np.eye(128, dtype=np.float32)
    wqT = np.ascontiguousarray(np.asarray(WQ, dtype=np.float32).T)
    wkT = np.ascontiguousarray(np.asarray(WK, dtype=np.float32).T)
    wvT = np.ascontiguousarray(np.asarray(WV, dtype=np.float32).T)
    notm = ~np.asarray(mask)  # True where attention is allowed
    in_maps = []
    for b in range(B):
        in_maps.append(
            {
                "qT": np.ascontiguousarray(np.asarray(query[b], dtype=np.float32).T),
                "kT": np.ascontiguousarray(np.asarray(key[b], dtype=np.float32).T),
                "vT": np.ascontiguousarray(np.asarray(value[b], dtype=np.float32).T),
                "notmT": np.ascontiguousarray(notm[b].T).view(np.uint8),
                "wqT": wqT,
                "wkT": wkT,
                "wvT": wvT,
                "ident": ident,
            }
        )
    res = bass_utils.run_bass_kernel_spmd(nc, in_maps, core_ids=list(range(NCORES)))
    out = np.stack([res.results[b]["out"] for b in range(B)], axis=0)
    return out


if __name__ == "__main__":
    rng = np.random.default_rng(0)
    q = rng.standard_normal((B, L, E), dtype=np.float32)
    k = rng.standard_normal((B, L, E), dtype=np.float32)
    v = rng.standard_normal((B, L, E), dtype=np.float32)
    m = rng.integers(0, 2, size=(B, L, L)).astype(bool)
    s = 1.0 / np.sqrt(E)
    wq = rng.uniform(-s, s, size=(H, E)).astype(np.float32)
    wk = rng.uniform(-s, s, size=(H, E)).astype(np.float32)
    wv = rng.uniform(-s, s, size=(H, E)).astype(np.float32)
    o = kernel(query=q, key=k, value=v, mask=m, WQ=wq, WK=wk, WV=wv)
    print(o.shape, o.dtype)
